# revision 19
# baseline (speedup 1.0000x reference)
"""Bass/Trainium2 kernel for FLAOperator(mode='gla') CPU-fallback scan.

Reference recurrence (per b, h, d lane, over t = 0..N-1):
    s_t = s_{t-1} + sigmoid(q_t * k_t + g_t) * v_t ;  y_t = s_t
i.e. y = cumsum over N of u, with u = sigmoid(q*k + g) * v  (pure elementwise).

Shapes: q,k,v,g,y all [B=2, H=16, N=4096, D=128] f32.

Strategy (8 NeuronCores, SPMD, no collectives):
  - Shard the 32 independent (b,h) recurrences: 4 per core.
  - Host-side prep: transpose each (b,h) slab to [D, N] and cast to bf16.
    The kernel is HBM-bound (the recurrence is elementwise), so bf16 I/O
    halves the traffic: 16 MiB in + 4 MiB out per core vs 40 MiB for f32.
    bf16 input rounding costs ~0.4% relative error on the cumsum (errors
    and signal both grow as sqrt(t)), well inside the 2e-2 gate.
  - SBUF layout [partition = d, free = n]: every DMA descriptor is a 2 KiB
    contiguous run per partition (full line rate, ~425 GB/s measured).
  - The whole recurrence is ONE DVE tensor_tensor_scan per tile (fp32
    internal state, so the accumulation itself is exact); tiles are
    half-sequences [128, 2048] chained via initial=prev[:, -1:].
  - All elementwise ops and scans run on DVE (Pool/gpsimd software TT ops
    slow concurrent DVE scans ~1.7x, so Pool is left idle); sigmoid on ACT.
    Two-stage software-pipelined emission hides the ACT round-trip.
    DMA issue: sync (q,k,v in), scalar (g in, y out).
"""

from contextlib import ExitStack

import ml_dtypes
import numpy as np

import concourse.bass as bass
import concourse.tile as tile
from concourse import bacc, mybir
from concourse.bass_utils import run_bass_kernel_spmd

B, H, N, D = 2, 16, 4096, 128
N_CORES = 8
BH = B * H                    # 32 independent recurrences
BH_PER_CORE = BH // N_CORES   # 4
P = 128                       # partitions (= D)
NQ = 2                        # half-sequence tiles per (b,h)
Q = N // NQ                   # 2048 columns per tile
F32 = mybir.dt.float32
BF16 = mybir.dt.bfloat16
BF16_NP = ml_dtypes.bfloat16

_PROGRAM = None       # cached compiled Bass program (module-level)
LAST_RESULTS = None   # BassKernelResults of the last run (for test harness)


def _build_program() -> bass.Bass:
    nc = bacc.Bacc("TRN2", debug=False, num_devices=N_CORES)

    q_d = nc.dram_tensor("q", [BH_PER_CORE, D, N], BF16, kind="ExternalInput").ap()
    k_d = nc.dram_tensor("k", [BH_PER_CORE, D, N], BF16, kind="ExternalInput").ap()
    v_d = nc.dram_tensor("v", [BH_PER_CORE, D, N], BF16, kind="ExternalInput").ap()
    g_d = nc.dram_tensor("g", [BH_PER_CORE, D, N], BF16, kind="ExternalInput").ap()
    y_d = nc.dram_tensor("y", [BH_PER_CORE, D, N], BF16, kind="ExternalOutput").ap()

    with tile.TileContext(nc) as tc, ExitStack() as ctx:
        const_pool = ctx.enter_context(tc.tile_pool(name="const", bufs=1))
        io_pool = ctx.enter_context(tc.tile_pool(name="io", bufs=6))
        ioq_pool = ctx.enter_context(tc.tile_pool(name="ioq", bufs=3))
        a_pool = ctx.enter_context(tc.tile_pool(name="a", bufs=4))
        y_pool = ctx.enter_context(tc.tile_pool(name="y", bufs=3))

        # Dummy sigmoid so the ACT function table loads during the
        # framework preamble instead of stalling the first real unit.
        warm = const_pool.tile([P, 2], BF16, tag="warm")
        nc.vector.memset(warm[:], 0.0)
        nc.scalar.activation(warm[:], warm[:], mybir.ActivationFunctionType.Sigmoid)

        # Software-pipelined over units (bh, start, len): stage 1 (mul, add,
        # sigmoid) runs one unit ahead of stage 2 (mul, scan, store) so the
        # ACT sigmoid hides under the next unit's DVE work — the DVE queue
        # is in-order, so emission order is schedule order. bh0 starts with
        # quarter-size segments so the first DMAs land (and DVE starts)
        # sooner; everything else runs at half-sequence granularity.
        HQ = Q // 2
        units = [(0, 0, HQ), (0, HQ, HQ), (0, 2 * HQ, Q)]
        units += [(bh, j * Q, Q) for bh in range(1, BH_PER_CORE - 1) for j in range(NQ)]
        # last bh tapers back to quarters so the final scan/store are small
        units += [(BH_PER_CORE - 1, 0, Q), (BH_PER_CORE - 1, Q, HQ),
                  (BH_PER_CORE - 1, Q + HQ, HQ)]
        stage1 = {}   # unit -> (vt, st) awaiting stage 2
        prev_y = {}   # bh -> (tile, len) of previous segment (scan chain)

        def emit_stage1(u):
            bh, st0, ln = u
            sl = slice(st0, st0 + ln)
            pool = ioq_pool if ln < Q else io_pool
            qt = pool.tile([P, ln], BF16, tag=f"q{ln}")
            kt = pool.tile([P, ln], BF16, tag=f"k{ln}")
            vt = pool.tile([P, ln], BF16, tag=f"v{ln}")
            gt = pool.tile([P, ln], BF16, tag=f"g{ln}")
            # sync queue carries only q,k (the stage-1 critical operands);
            # g and v (needed one stage later) ride the scalar queue
            nc.sync.dma_start(out=qt[:], in_=q_d[bh, :, sl])
            nc.sync.dma_start(out=kt[:], in_=k_d[bh, :, sl])
            nc.scalar.dma_start(out=gt[:], in_=g_d[bh, :, sl])
            nc.scalar.dma_start(out=vt[:], in_=v_d[bh, :, sl])
            at = a_pool.tile([P, ln], BF16, tag=f"a{ln}")
            nc.vector.tensor_mul(at[:], qt[:], kt[:])
            nc.vector.tensor_add(at[:], at[:], gt[:])
            nc.scalar.activation(
                at[:], at[:], mybir.ActivationFunctionType.Sigmoid
            )
            stage1[u] = (vt, at)

        def emit_stage2(u):
            bh, st0, ln = u
            vt, st = stage1.pop(u)
            sl = slice(st0, st0 + ln)
            ut = a_pool.tile([P, ln], BF16, tag=f"u{ln}")
            nc.vector.tensor_mul(ut[:], st[:], vt[:])
            yt = y_pool.tile([P, ln], BF16, tag=f"y{ln}")
            if bh in prev_y:
                pt, pl = prev_y[bh]
                init = pt[:, pl - 1 : pl]
            else:
                init = 0.0
            nc.vector.tensor_tensor_scan(
                out=yt[:], data0=ut[:], data1=ut[:], initial=init,
                op0=mybir.AluOpType.add, op1=mybir.AluOpType.bypass,
            )
            prev_y[bh] = (yt, ln)
            nc.scalar.dma_start(out=y_d[bh, :, sl], in_=yt[:])

        LAG = 2  # stage-2 lag in units: deeper shadow for the ACT sigmoid
        for idx, u in enumerate(units):
            emit_stage1(u)
            if idx >= LAG:
                emit_stage2(units[idx - LAG])
        for u in units[-LAG:]:
            emit_stage2(u)

    nc.compile()  # bacc backend: wait legalization, reg alloc, nop fusion
    return nc


def kernel(q: np.ndarray, k: np.ndarray, v: np.ndarray, g: np.ndarray) -> np.ndarray:
    global _PROGRAM, LAST_RESULTS
    if _PROGRAM is None:
        _PROGRAM = _build_program()

    def prep(x):
        # [B, H, N, D] f32 -> [BH, D, N] bf16 (time-major per (b,h,d) lane)
        x = np.asarray(x, dtype=np.float32).reshape(BH, N, D)
        return x.transpose(0, 2, 1).astype(BF16_NP)

    qp, kp, vp, gp = prep(q), prep(k), prep(v), prep(g)
    in_maps = []
    for i in range(N_CORES):
        s = slice(i * BH_PER_CORE, (i + 1) * BH_PER_CORE)
        in_maps.append({"q": qp[s], "k": kp[s], "v": vp[s], "g": gp[s]})

    LAST_RESULTS = run_bass_kernel_spmd(_PROGRAM, in_maps, core_ids=list(range(N_CORES)))
    y = np.concatenate([r["y"] for r in LAST_RESULTS.results], axis=0)  # [BH, D, N]
    return y.transpose(0, 2, 1).astype(np.float32).reshape(B, H, N, D)


# revision 25
# speedup vs baseline: 1.1817x; 1.1817x over previous
"""Bass/Trainium2 kernel for FLAOperator(mode='gla') CPU-fallback scan.

Reference recurrence (per b, h, d lane, over t = 0..N-1):
    s_t = s_{t-1} + sigmoid(q_t * k_t + g_t) * v_t ;  y_t = s_t
i.e. y = cumsum over N of u, with u = sigmoid(q*k + g) * v  (pure elementwise).

Shapes: q,k,v,g,y all [B=2, H=16, N=4096, D=128] f32.

Strategy (8 NeuronCores, SPMD, no collectives):
  - Shard the 32 independent (b,h) recurrences: 4 per core.
  - Host-side prep: transpose each (b,h) slab to [D, N] and cast to bf16.
    The kernel is HBM-bound (the recurrence is elementwise), so bf16 I/O
    halves the traffic: 16 MiB in + 4 MiB out per core vs 40 MiB for f32.
    bf16 input rounding costs ~0.4% relative error on the cumsum (errors
    and signal both grow as sqrt(t)), well inside the 2e-2 gate.
  - SBUF layout [partition = d, free = n]: every DMA descriptor is a 2 KiB
    contiguous run per partition (full line rate, ~425 GB/s measured).
  - The whole recurrence is ONE DVE tensor_tensor_scan per tile (fp32
    internal state, so the accumulation itself is exact); tiles are
    half-sequences [128, 2048] chained via initial=prev[:, -1:].
  - All elementwise ops and scans run on DVE (Pool/gpsimd software TT ops
    slow concurrent DVE scans ~1.7x, so Pool is left idle); sigmoid on ACT.
    Two-stage software-pipelined emission hides the ACT round-trip.
    DMA issue: sync (q,k,v in), scalar (g in, y out).
"""

from contextlib import ExitStack

import ml_dtypes
import numpy as np

import concourse.bass as bass
import concourse.tile as tile
from concourse import bacc, mybir
from concourse.bass_utils import run_bass_kernel_spmd

B, H, N, D = 2, 16, 4096, 128
N_CORES = 8
BH = B * H                    # 32 independent recurrences
BH_PER_CORE = BH // N_CORES   # 4
P = 128                       # partitions (= D)
NQ = 2                        # half-sequence tiles per (b,h)
Q = N // NQ                   # 2048 columns per tile
F32 = mybir.dt.float32
BF16 = mybir.dt.bfloat16
BF16_NP = ml_dtypes.bfloat16

_PROGRAM = None       # cached compiled Bass program (module-level)
LAST_RESULTS = None   # BassKernelResults of the last run (for test harness)


def _build_program() -> bass.Bass:
    nc = bacc.Bacc("TRN2", debug=False, num_devices=N_CORES)

    q_d = nc.dram_tensor("q", [BH_PER_CORE, D, N], BF16, kind="ExternalInput").ap()
    k_d = nc.dram_tensor("k", [BH_PER_CORE, D, N], BF16, kind="ExternalInput").ap()
    v_d = nc.dram_tensor("v", [BH_PER_CORE, D, N], BF16, kind="ExternalInput").ap()
    g_d = nc.dram_tensor("g", [BH_PER_CORE, D, N], BF16, kind="ExternalInput").ap()
    y_d = nc.dram_tensor("y", [BH_PER_CORE, D, N], BF16, kind="ExternalOutput").ap()

    with tile.TileContext(nc) as tc, ExitStack() as ctx:
        const_pool = ctx.enter_context(tc.tile_pool(name="const", bufs=1))
        io_pool = ctx.enter_context(tc.tile_pool(name="io", bufs=6))
        ioq_pool = ctx.enter_context(tc.tile_pool(name="ioq", bufs=2))
        a_pool = ctx.enter_context(tc.tile_pool(name="a", bufs=3))
        y_pool = ctx.enter_context(tc.tile_pool(name="y", bufs=3))

        # Dummy sigmoid so the ACT function table loads during the
        # framework preamble instead of stalling the first real unit.
        warm = const_pool.tile([P, 2], BF16, tag="warm")
        nc.vector.memset(warm[:], 0.0)
        nc.scalar.activation(warm[:], warm[:], mybir.ActivationFunctionType.Sigmoid)

        # Software-pipelined over units (bh, start, len): stage 1 (mul, add,
        # sigmoid) runs one unit ahead of stage 2 (mul, scan, store) so the
        # ACT sigmoid hides under the next unit's DVE work — the DVE queue
        # is in-order, so emission order is schedule order. bh0 starts with
        # quarter-size segments so the first DMAs land (and DVE starts)
        # sooner; everything else runs at half-sequence granularity.
        HQ = Q // 2
        units = [(0, 0, HQ), (0, HQ, HQ), (0, 2 * HQ, Q)]
        units += [(bh, j * Q, Q) for bh in range(1, BH_PER_CORE - 1) for j in range(NQ)]
        # taper the last bh to quarters so the final scan + store are small
        units += [(BH_PER_CORE - 1, 0, Q), (BH_PER_CORE - 1, Q, HQ),
                  (BH_PER_CORE - 1, Q + HQ, HQ)]
        stage1 = {}   # unit -> (vt, st) awaiting stage 2
        prev_y = {}   # bh -> (tile, len) of previous segment (scan chain)

        def emit_stage1(u):
            bh, st0, ln = u
            sl = slice(st0, st0 + ln)
            pool = ioq_pool if ln < Q else io_pool
            qt = pool.tile([P, ln], BF16, tag=f"q{ln}")
            kt = pool.tile([P, ln], BF16, tag=f"k{ln}")
            vt = pool.tile([P, ln], BF16, tag=f"v{ln}")
            gt = pool.tile([P, ln], BF16, tag=f"g{ln}")
            # The very first unit rides entirely on the sync queue so its
            # q/k packets get all 16 DMA engines (no competing g-stream);
            # steady state keeps g on the scalar queue (with y-outs).
            first = (bh, st0) == (0, 0)
            nc.sync.dma_start(out=qt[:], in_=q_d[bh, :, sl])
            nc.sync.dma_start(out=kt[:], in_=k_d[bh, :, sl])
            (nc.sync if first else nc.scalar).dma_start(out=gt[:], in_=g_d[bh, :, sl])
            nc.sync.dma_start(out=vt[:], in_=v_d[bh, :, sl])
            at = a_pool.tile([P, ln], BF16, tag=f"a{ln}")
            nc.vector.tensor_mul(at[:], qt[:], kt[:])
            nc.vector.tensor_add(at[:], at[:], gt[:])
            nc.scalar.activation(
                at[:], at[:], mybir.ActivationFunctionType.Sigmoid
            )
            stage1[u] = (vt, at)

        def emit_stage2(u):
            bh, st0, ln = u
            vt, st = stage1.pop(u)
            sl = slice(st0, st0 + ln)
            ut = a_pool.tile([P, ln], BF16, tag=f"u{ln}")
            nc.vector.tensor_mul(ut[:], st[:], vt[:])
            yt = y_pool.tile([P, ln], BF16, tag=f"y{ln}")
            if bh in prev_y:
                pt, pl = prev_y[bh]
                init = pt[:, pl - 1 : pl]
            else:
                init = 0.0
            nc.vector.tensor_tensor_scan(
                out=yt[:], data0=ut[:], data1=ut[:], initial=init,
                op0=mybir.AluOpType.add, op1=mybir.AluOpType.bypass,
            )
            prev_y[bh] = (yt, ln)
            nc.scalar.dma_start(out=y_d[bh, :, sl], in_=yt[:])

        for idx, u in enumerate(units):
            emit_stage1(u)
            if idx >= 1:
                emit_stage2(units[idx - 1])
        emit_stage2(units[-1])

    nc.compile()  # bacc backend: wait legalization, reg alloc, nop fusion
    return nc


def kernel(q: np.ndarray, k: np.ndarray, v: np.ndarray, g: np.ndarray) -> np.ndarray:
    global _PROGRAM, LAST_RESULTS
    if _PROGRAM is None:
        _PROGRAM = _build_program()

    def prep(x):
        # [B, H, N, D] f32 -> [BH, D, N] bf16 (time-major per (b,h,d) lane)
        x = np.asarray(x, dtype=np.float32).reshape(BH, N, D)
        return x.transpose(0, 2, 1).astype(BF16_NP)

    qp, kp, vp, gp = prep(q), prep(k), prep(v), prep(g)
    in_maps = []
    for i in range(N_CORES):
        s = slice(i * BH_PER_CORE, (i + 1) * BH_PER_CORE)
        in_maps.append({"q": qp[s], "k": kp[s], "v": vp[s], "g": gp[s]})

    LAST_RESULTS = run_bass_kernel_spmd(_PROGRAM, in_maps, core_ids=list(range(N_CORES)))
    y = np.concatenate([r["y"] for r in LAST_RESULTS.results], axis=0)  # [BH, D, N]
    return y.transpose(0, 2, 1).astype(np.float32).reshape(B, H, N, D)


# revision 30
# speedup vs baseline: 1.1896x; 1.0067x over previous
"""Bass/Trainium2 kernel for FLAOperator(mode='gla') CPU-fallback scan.

Reference recurrence (per b, h, d lane, over t = 0..N-1):
    s_t = s_{t-1} + sigmoid(q_t * k_t + g_t) * v_t ;  y_t = s_t
i.e. y = cumsum over N of u, with u = sigmoid(q*k + g) * v  (pure elementwise).

Shapes: q,k,v,g,y all [B=2, H=16, N=4096, D=128] f32.

Strategy (8 NeuronCores, SPMD, no collectives):
  - Shard the 32 independent (b,h) recurrences: 4 per core.
  - Host-side prep: transpose each (b,h) slab to [D, N] and cast to bf16.
    The kernel is HBM-bound (the recurrence is elementwise), so bf16 I/O
    halves the traffic: 16 MiB in + 4 MiB out per core vs 40 MiB for f32.
    bf16 input rounding costs ~0.4% relative error on the cumsum (errors
    and signal both grow as sqrt(t)), well inside the 2e-2 gate.
  - SBUF layout [partition = d, free = n]: every DMA descriptor is a 2 KiB
    contiguous run per partition (full line rate, ~425 GB/s measured).
  - The whole recurrence is ONE DVE tensor_tensor_scan per tile (fp32
    internal state, so the accumulation itself is exact); tiles are
    half-sequences [128, 2048] chained via initial=prev[:, -1:].
  - All elementwise ops and scans run on DVE (Pool/gpsimd software TT ops
    slow concurrent DVE scans ~1.7x, so Pool is left idle); sigmoid on ACT.
    Two-stage software-pipelined emission hides the ACT round-trip.
    DMA issue: sync (q,k,v in), scalar (g in, y out).
"""

from contextlib import ExitStack

import ml_dtypes
import numpy as np

import concourse.bass as bass
import concourse.tile as tile
from concourse import bacc, mybir
from concourse.bass_utils import run_bass_kernel_spmd

B, H, N, D = 2, 16, 4096, 128
N_CORES = 8
BH = B * H                    # 32 independent recurrences
BH_PER_CORE = BH // N_CORES   # 4
P = 128                       # partitions (= D)
NQ = 2                        # half-sequence tiles per (b,h)
Q = N // NQ                   # 2048 columns per tile
F32 = mybir.dt.float32
BF16 = mybir.dt.bfloat16
BF16_NP = ml_dtypes.bfloat16

_PROGRAM = None       # cached compiled Bass program (module-level)
LAST_RESULTS = None   # BassKernelResults of the last run (for test harness)


def _build_program() -> bass.Bass:
    nc = bacc.Bacc("TRN2", debug=False, num_devices=N_CORES)

    # q,k,g,v packed on the host into one tensor: one DMA (and one
    # completion semaphore) per unit instead of four
    x_d = nc.dram_tensor("x", [BH_PER_CORE, 4, D, N], BF16, kind="ExternalInput").ap()
    y_d = nc.dram_tensor("y", [BH_PER_CORE, D, N], BF16, kind="ExternalOutput").ap()

    with tile.TileContext(nc) as tc, ExitStack() as ctx:
        const_pool = ctx.enter_context(tc.tile_pool(name="const", bufs=1))
        io_pool = ctx.enter_context(tc.tile_pool(name="io", bufs=6))
        ioq_pool = ctx.enter_context(tc.tile_pool(name="ioq", bufs=2))
        a_pool = ctx.enter_context(tc.tile_pool(name="a", bufs=3))
        y_pool = ctx.enter_context(tc.tile_pool(name="y", bufs=3))

        # Dummy sigmoid so the ACT function table loads during the
        # framework preamble instead of stalling the first real unit.
        warm = const_pool.tile([P, 2], BF16, tag="warm")
        nc.vector.memset(warm[:], 0.0)
        nc.scalar.activation(warm[:], warm[:], mybir.ActivationFunctionType.Sigmoid)

        # Software-pipelined over units (bh, start, len): stage 1 (mul, add,
        # sigmoid) runs one unit ahead of stage 2 (mul, scan, store) so the
        # ACT sigmoid hides under the next unit's DVE work — the DVE queue
        # is in-order, so emission order is schedule order. bh0 starts with
        # quarter-size segments so the first DMAs land (and DVE starts)
        # sooner; everything else runs at half-sequence granularity.
        HQ = Q // 2
        units = [(0, 0, HQ), (0, HQ, HQ), (0, 2 * HQ, Q)]
        units += [(bh, j * Q, Q) for bh in range(1, BH_PER_CORE) for j in range(NQ)]
        stage1 = {}   # unit -> (vt, st) awaiting stage 2
        prev_y = {}   # bh -> (tile, len) of previous segment (scan chain)

        def emit_stage1(u):
            bh, st0, ln = u
            sl = slice(st0, st0 + ln)
            pool = ioq_pool if ln < Q else io_pool
            xt = pool.tile([P, 4 * ln], BF16, tag=f"x{ln}")
            x4 = xt[:].rearrange("d (t n) -> d t n", t=4)
            nc.sync.dma_start(
                out=x4, in_=x_d[bh, :, :, sl].rearrange("t d n -> d t n")
            )
            at = a_pool.tile([P, ln], BF16, tag=f"a{ln}")
            nc.vector.tensor_mul(at[:], x4[:, 0, :], x4[:, 1, :])
            nc.vector.tensor_add(at[:], at[:], x4[:, 2, :])
            nc.scalar.activation(
                at[:], at[:], mybir.ActivationFunctionType.Sigmoid
            )
            stage1[u] = (x4, at)

        def emit_stage2(u):
            bh, st0, ln = u
            x4, st = stage1.pop(u)
            sl = slice(st0, st0 + ln)
            ut = a_pool.tile([P, ln], BF16, tag=f"u{ln}")
            nc.vector.tensor_mul(ut[:], st[:], x4[:, 3, :])
            yt = y_pool.tile([P, ln], BF16, tag=f"y{ln}")
            if bh in prev_y:
                pt, pl = prev_y[bh]
                init = pt[:, pl - 1 : pl]
            else:
                init = 0.0
            nc.vector.tensor_tensor_scan(
                out=yt[:], data0=ut[:], data1=ut[:], initial=init,
                op0=mybir.AluOpType.add, op1=mybir.AluOpType.bypass,
            )
            prev_y[bh] = (yt, ln)
            nc.scalar.dma_start(out=y_d[bh, :, sl], in_=yt[:])

        for idx, u in enumerate(units):
            emit_stage1(u)
            if idx >= 1:
                emit_stage2(units[idx - 1])
        emit_stage2(units[-1])

    nc.compile()  # bacc backend: wait legalization, reg alloc, nop fusion
    return nc


def kernel(q: np.ndarray, k: np.ndarray, v: np.ndarray, g: np.ndarray) -> np.ndarray:
    global _PROGRAM, LAST_RESULTS
    if _PROGRAM is None:
        _PROGRAM = _build_program()

    def prep(x):
        # [B, H, N, D] f32 -> [BH, D, N] bf16 (time-major per (b,h,d) lane)
        x = np.asarray(x, dtype=np.float32).reshape(BH, N, D)
        return x.transpose(0, 2, 1).astype(BF16_NP)

    # pack [BH, 4, D, N]: t=0:q, 1:k, 2:g, 3:v (matches kernel indices)
    xp = np.stack([prep(q), prep(k), prep(g), prep(v)], axis=1)
    in_maps = []
    for i in range(N_CORES):
        s = slice(i * BH_PER_CORE, (i + 1) * BH_PER_CORE)
        in_maps.append({"x": xp[s]})

    LAST_RESULTS = run_bass_kernel_spmd(_PROGRAM, in_maps, core_ids=list(range(N_CORES)))
    y = np.concatenate([r["y"] for r in LAST_RESULTS.results], axis=0)  # [BH, D, N]
    return y.transpose(0, 2, 1).astype(np.float32).reshape(B, H, N, D)


# revision 32
# speedup vs baseline: 1.1975x; 1.0066x over previous
"""Bass/Trainium2 kernel for FLAOperator(mode='gla') CPU-fallback scan.

Reference recurrence (per b, h, d lane, over t = 0..N-1):
    s_t = s_{t-1} + sigmoid(q_t * k_t + g_t) * v_t ;  y_t = s_t
i.e. y = cumsum over N of u, with u = sigmoid(q*k + g) * v  (pure elementwise).

Shapes: q,k,v,g,y all [B=2, H=16, N=4096, D=128] f32.

Strategy (8 NeuronCores, SPMD, no collectives):
  - Shard the 32 independent (b,h) recurrences: 4 per core.
  - Host-side prep: transpose each (b,h) slab to [D, N] and cast to bf16.
    The kernel is HBM-bound (the recurrence is elementwise), so bf16 I/O
    halves the traffic: 16 MiB in + 4 MiB out per core vs 40 MiB for f32.
    bf16 input rounding costs ~0.4% relative error on the cumsum (errors
    and signal both grow as sqrt(t)), well inside the 2e-2 gate.
  - SBUF layout [partition = d, free = n]: every DMA descriptor is a 2 KiB
    contiguous run per partition (full line rate, ~425 GB/s measured).
  - The whole recurrence is ONE DVE tensor_tensor_scan per tile (fp32
    internal state, so the accumulation itself is exact); tiles are
    half-sequences [128, 2048] chained via initial=prev[:, -1:].
  - All elementwise ops and scans run on DVE (Pool/gpsimd software TT ops
    slow concurrent DVE scans ~1.7x, so Pool is left idle); sigmoid on ACT.
    Two-stage software-pipelined emission hides the ACT round-trip.
    DMA issue: sync (q,k,v in), scalar (g in, y out).
"""

from contextlib import ExitStack

import ml_dtypes
import numpy as np

import concourse.bass as bass
import concourse.tile as tile
from concourse import bacc, mybir
from concourse.bass_utils import run_bass_kernel_spmd

B, H, N, D = 2, 16, 4096, 128
N_CORES = 8
BH = B * H                    # 32 independent recurrences
BH_PER_CORE = BH // N_CORES   # 4
P = 128                       # partitions (= D)
NQ = 2                        # half-sequence tiles per (b,h)
Q = N // NQ                   # 2048 columns per tile
F32 = mybir.dt.float32
BF16 = mybir.dt.bfloat16
BF16_NP = ml_dtypes.bfloat16

_PROGRAM = None       # cached compiled Bass program (module-level)
LAST_RESULTS = None   # BassKernelResults of the last run (for test harness)


def _build_program() -> bass.Bass:
    nc = bacc.Bacc("TRN2", debug=False, num_devices=N_CORES)

    q_d = nc.dram_tensor("q", [BH_PER_CORE, D, N], BF16, kind="ExternalInput").ap()
    k_d = nc.dram_tensor("k", [BH_PER_CORE, D, N], BF16, kind="ExternalInput").ap()
    v_d = nc.dram_tensor("v", [BH_PER_CORE, D, N], BF16, kind="ExternalInput").ap()
    g_d = nc.dram_tensor("g", [BH_PER_CORE, D, N], BF16, kind="ExternalInput").ap()
    y_d = nc.dram_tensor("y", [BH_PER_CORE, D, N], BF16, kind="ExternalOutput").ap()

    with tile.TileContext(nc) as tc, ExitStack() as ctx:
        const_pool = ctx.enter_context(tc.tile_pool(name="const", bufs=1))
        io_pool = ctx.enter_context(tc.tile_pool(name="io", bufs=6))
        ioq_pool = ctx.enter_context(tc.tile_pool(name="ioq", bufs=2))
        a_pool = ctx.enter_context(tc.tile_pool(name="a", bufs=3))
        y_pool = ctx.enter_context(tc.tile_pool(name="y", bufs=3))

        # Dummy sigmoid so the ACT function table loads during the
        # framework preamble instead of stalling the first real unit.
        warm = const_pool.tile([P, 2], BF16, tag="warm")
        nc.vector.memset(warm[:], 0.0)
        nc.scalar.activation(warm[:], warm[:], mybir.ActivationFunctionType.Sigmoid)

        # Software-pipelined over units (bh, start, len): stage 1 (mul, add,
        # sigmoid) runs one unit ahead of stage 2 (mul, scan, store) so the
        # ACT sigmoid hides under the next unit's DVE work — the DVE queue
        # is in-order, so emission order is schedule order. bh0 starts with
        # quarter-size segments so the first DMAs land (and DVE starts)
        # sooner; everything else runs at half-sequence granularity.
        HQ = Q // 2
        units = [(0, 0, HQ), (0, HQ, HQ), (0, 2 * HQ, Q)]
        units += [(bh, j * Q, Q) for bh in range(1, BH_PER_CORE) for j in range(NQ)]
        stage1 = {}   # unit -> (vt, st) awaiting stage 2
        prev_y = {}   # bh -> (tile, len) of previous segment (scan chain)

        # Hoist the q/k issues for the first units ahead of any v issue so
        # the early DVE muls never wait behind v transfers on the sync queue.
        preQK = {}
        for u in units[:3]:
            bh, st0, ln = u
            sl = slice(st0, st0 + ln)
            pool = ioq_pool if ln < Q else io_pool
            qt = pool.tile([P, ln], BF16, tag=f"q{ln}")
            kt = pool.tile([P, ln], BF16, tag=f"k{ln}")
            nc.sync.dma_start(out=qt[:], in_=q_d[bh, :, sl])
            nc.sync.dma_start(out=kt[:], in_=k_d[bh, :, sl])
            preQK[u] = (qt, kt)

        def emit_stage1(u):
            bh, st0, ln = u
            sl = slice(st0, st0 + ln)
            pool = ioq_pool if ln < Q else io_pool
            if u in preQK:
                qt, kt = preQK.pop(u)
            else:
                qt = pool.tile([P, ln], BF16, tag=f"q{ln}")
                kt = pool.tile([P, ln], BF16, tag=f"k{ln}")
                nc.sync.dma_start(out=qt[:], in_=q_d[bh, :, sl])
                nc.sync.dma_start(out=kt[:], in_=k_d[bh, :, sl])
            vt = pool.tile([P, ln], BF16, tag=f"v{ln}")
            gt = pool.tile([P, ln], BF16, tag=f"g{ln}")
            nc.sync.dma_start(out=vt[:], in_=v_d[bh, :, sl])
            nc.scalar.dma_start(out=gt[:], in_=g_d[bh, :, sl])
            at = a_pool.tile([P, ln], BF16, tag=f"a{ln}")
            nc.vector.tensor_mul(at[:], qt[:], kt[:])
            nc.vector.tensor_add(at[:], at[:], gt[:])
            nc.scalar.activation(
                at[:], at[:], mybir.ActivationFunctionType.Sigmoid
            )
            stage1[u] = (vt, at)

        def emit_stage2(u):
            bh, st0, ln = u
            vt, st = stage1.pop(u)
            sl = slice(st0, st0 + ln)
            ut = a_pool.tile([P, ln], BF16, tag=f"u{ln}")
            nc.vector.tensor_mul(ut[:], st[:], vt[:])
            yt = y_pool.tile([P, ln], BF16, tag=f"y{ln}")
            if bh in prev_y:
                pt, pl = prev_y[bh]
                init = pt[:, pl - 1 : pl]
            else:
                init = 0.0
            nc.vector.tensor_tensor_scan(
                out=yt[:], data0=ut[:], data1=ut[:], initial=init,
                op0=mybir.AluOpType.add, op1=mybir.AluOpType.bypass,
            )
            prev_y[bh] = (yt, ln)
            nc.scalar.dma_start(out=y_d[bh, :, sl], in_=yt[:])

        for idx, u in enumerate(units):
            emit_stage1(u)
            if idx >= 1:
                emit_stage2(units[idx - 1])
        emit_stage2(units[-1])

    nc.compile()  # bacc backend: wait legalization, reg alloc, nop fusion
    return nc


def kernel(q: np.ndarray, k: np.ndarray, v: np.ndarray, g: np.ndarray) -> np.ndarray:
    global _PROGRAM, LAST_RESULTS
    if _PROGRAM is None:
        _PROGRAM = _build_program()

    def prep(x):
        # [B, H, N, D] f32 -> [BH, D, N] bf16 (time-major per (b,h,d) lane)
        x = np.asarray(x, dtype=np.float32).reshape(BH, N, D)
        return x.transpose(0, 2, 1).astype(BF16_NP)

    qp, kp, vp, gp = prep(q), prep(k), prep(v), prep(g)
    in_maps = []
    for i in range(N_CORES):
        s = slice(i * BH_PER_CORE, (i + 1) * BH_PER_CORE)
        in_maps.append({"q": qp[s], "k": kp[s], "v": vp[s], "g": gp[s]})

    LAST_RESULTS = run_bass_kernel_spmd(_PROGRAM, in_maps, core_ids=list(range(N_CORES)))
    y = np.concatenate([r["y"] for r in LAST_RESULTS.results], axis=0)  # [BH, D, N]
    return y.transpose(0, 2, 1).astype(np.float32).reshape(B, H, N, D)
